# revision 23
# baseline (speedup 1.0000x reference)
"""Trainium2 Bass kernel for a pre-LN transformer block (B=2, T=2048, C=1024, H=16).

Strategy (8 NeuronCores, SPMD), v4:
  - Tensor-parallel over heads for attention: core c owns heads {2c, 2c+1}.
  - LN1 stats computed locally per chunk (mean/meansq via ones-matmuls into
    PSUM); the LN1 affine is folded post-matmul into qkv.
  - Batch-1 qkv uses host-swapped weight columns (wqk2) so both batches take
    a single full-partition matmul (v3 used two half-partition matmuls).
  - Chunk-ahead software pipeline: chunk(jj+1) qkv matmuls are emitted before
    attn(jj), so the PE has backlog while chunk(jj+1)'s LN fold chain
    (Scalar/Vector/GpSimd) resolves.
  - Row ownership is 64-token interleaved: core c owns rows [64c, 64c+64) of
    every (batch, jj) 512-chunk, so each per-jj A2A feeds every core and
    token tile tt unlocks at assembly jj=tt.
  - MLP fc is split into two half-token passes: fc(tt0,tt1) is emitted right
    after attn(3), filling the PE while the last (4-way split) A2A flies;
    fc(tt2,tt3) runs after assembly(3)+proj(3)+ln2(3). fc weights stream
    twice (extra 8MB HBM, hidden under compute).
  - exp batched 2 key-tiles per ACT ([128,1024] PSUM read); causal masks
    multiplied on VectorE (bf16 2x).
  - Startup: small constants packed into two tensors (cpack/rpack); weights
    DMA'd as single large transfers; critical-path loads first.

PSUM budget (8 banks): tag "w2" = 3 x [128,1024] f32 (6 banks) +
tag "po" = 2 x [128,512] f32 (2 banks).
"""

import numpy as np
import ml_dtypes

from concourse import bass, bacc, tile, mybir, bass_utils

BF16 = mybir.dt.bfloat16
F32 = mybir.dt.float32
F32R = mybir.dt.float32r
I32 = mybir.dt.int32
QK3 = 0x5F375A86  # rsqrt magic
AX = mybir.AxisListType
OP = mybir.AluOpType
AF = mybir.ActivationFunctionType

B, T, C, H, HD = 2, 2048, 1024, 16, 64
NCORES = 8
BT = B * T                  # 4096 global tokens
RPC = BT // NCORES          # 512 rows per core
NCH = BT // 512             # 8 token chunks of 512
CB = C // 128               # 8 contraction blocks
HT = 4 * C // 128           # 32 hidden tiles
EPS = 1e-5

_cache = {}


def build():
    nc = bacc.Bacc("TRN2", target_bir_lowering=False, debug=False, num_devices=NCORES)

    def din(name, shape, dt=BF16):
        return nc.dram_tensor(name, list(shape), dt, kind="ExternalInput").ap()

    xT8 = din("xT8", [NCH, 128, 8 * 512])                 # x transposed, chunked
    wqka = din("wqka", [128, CB * 256])                   # [q_h0|q_h1|k_h0|k_h1] per cb
    wqkb = din("wqkb", [128, CB * 256])                   # head-swapped for batch 1
    wva = din("wva", [128, CB * 128])                     # [v_h0|v_h1] per cb
    cpack = din("cpack", [128, 42], F32)  # bq0,bq1,bk0,bk1,cq0,cq1,ck0,ck1,bv,cv,bfc(32)
    rpack = din("rpack", [1, 128 + 2 * C], F32R)          # onesr | bproj | bfcp
    x2r = din("x2r", [128, 4 * C], F32)                   # own residual rows, tt-major
    wproj = din("wproj", [128, 4 * 2048])
    wfc = din("wfc", [8, 128, 8 * 512])                   # htg -> [cb | 4 ht cols]
    wfcp = din("wfcp", [HT, 128, 1024])
    maskd = din("maskd", [128, 4 * 512])                  # 0/1 causal diag masks
    ident = din("ident", [128, 128])
    out_rows = nc.dram_tensor("out_rows", [4, 128, C], F32, kind="ExternalOutput").ap()

    with tile.TileContext(nc) as tc:
        with tc.tile_pool(name="persist", bufs=1) as pp, \
             tc.tile_pool(name="work", bufs=2) as wk, \
             tc.tile_pool(name="psum", bufs=1, space="PSUM") as ps, \
             tc.tile_pool(name="dram", bufs=1, space="DRAM") as dram:

            # ---------- critical-path loads first ----------
            ones_bf = pp.tile([128, 1], BF16, tag="ones_bf")
            nc.vector.memset(ones_bf[:], 1.0 / C)
            xta00 = None  # loaded in-pipeline via load_xta
            xta01 = None
            wqk_all = pp.tile([128, CB * 256], BF16, tag="wqk_all")
            nc.scalar.dma_start(wqk_all[:], wqka[:])
            wqk2_all = pp.tile([128, CB * 256], BF16, tag="wqk2_all")
            nc.gpsimd.dma_start(wqk2_all[:], wqkb[:])
            wv_all = pp.tile([128, CB * 128], BF16, tag="wv_all")
            nc.gpsimd.dma_start(wv_all[:], wva[:])
            wqk_sb = [wqk_all[:, 256 * cb:256 * (cb + 1)] for cb in range(CB)]
            wqk2_sb = [wqk2_all[:, 256 * cb:256 * (cb + 1)] for cb in range(CB)]
            wv_sb = [wv_all[:, 128 * cb:128 * (cb + 1)] for cb in range(CB)]

            cp = pp.tile([128, 42], F32, tag="cp")
            nc.gpsimd.dma_start(cp[:], cpack[:])
            rp = pp.tile([1, 128 + 2 * C], F32R, tag="rp")
            nc.gpsimd.dma_start(rp[:], rpack[:])
            bq_sb = [cp[:, v:v + 1] for v in range(2)]
            bk_sb = [cp[:, 2 + v:3 + v] for v in range(2)]
            cq_sb = [cp[:, 4 + v:5 + v] for v in range(2)]
            ck_sb = [cp[:, 6 + v:7 + v] for v in range(2)]
            bv_sb = cp[:, 8:9]
            cv_sb = cp[:, 9:10]
            bfc_sb = cp[:, 10:42]
            ones_row = rp[:, 0:128]
            bproj_sb = rp[:, 128:128 + C]
            bfcp_sb = rp[:, 128 + C:128 + 2 * C]

            # dummy collective at t~0 absorbs the first-collective mesh
            # rendezvous (~60us) off the critical path
            wdum = pp.tile([1, 16], BF16, tag="wdum")
            nc.vector.memset(wdum[:], 0.0)
            ib_d = dram.tile([8, 1, 2], BF16, tag="ib_d", name="ib_d")
            ob_d = dram.tile([8, 1, 2], BF16, tag="ob_d", name="ob_d")
            nc.gpsimd.dma_start(ib_d[:].rearrange("a b c -> b (a c)"), wdum[:])
            nc.gpsimd.collective_compute(
                "AllToAll", OP.bypass, ins=[ib_d.opt()], outs=[ob_d.opt()],
                replica_groups=[list(range(NCORES))],
            )

            eps1 = pp.tile([1, 1], F32, tag="eps1")
            nc.vector.memset(eps1[:], EPS)
            eps128 = pp.tile([128, 1], F32, tag="eps128")
            nc.vector.memset(eps128[:], EPS)
            mskbig = pp.tile([128, 4 * 512], BF16, tag="mskbig")
            nc.scalar.dma_start(mskbig[:], maskd[:])
            msk = [mskbig[:, 512 * m:512 * (m + 1)] for m in range(4)]
            idn = pp.tile([128, 128], BF16, tag="idn")
            nc.scalar.dma_start(idn[:], ident[:])

            # persistent activation tensors, head-major:
            qt = [pp.tile([128, T], BF16, tag=f"qt{X}", name=f"qt{X}") for X in range(2)]
            kt_ = [pp.tile([128, T], BF16, tag=f"kt{X}", name=f"ktt{X}") for X in range(2)]
            # v transposed, rows = key position within its 128-tile:
            # v1big[b][:, 130*kti + 65*h + d], col 65*h+64 = ones (denominator)
            v1big = [pp.tile([128, 16 * 130], BF16, tag=f"v1b{b}", name=f"v1b{b}")
                     for b in range(B)]
            for b in range(B):
                vr = v1big[b].rearrange("p (k h d) -> p k h d", k=16, h=2)
                nc.vector.memset(vr[:, :, :, 64:65], 1.0)

            # residual rows + A2A-assembled yT
            x2big = pp.tile([128, 4 * C], F32, tag="x2big")
            x2 = [x2big[:, C * tt:C * (tt + 1)] for tt in range(4)]
            yTbig = pp.tile([128, 8 * 512], BF16, tag="yTbig")
            wpjbig = pp.tile([128, 4 * 2048], BF16, tag="wpjbig")
            wpj = [wpjbig[:, 2048 * q:2048 * (q + 1)] for q in range(4)]
            ln2Tbig = pp.tile([128, 8 * 512], BF16, tag="ln2Tbig")
            ghT = [pp.tile([128, 512], BF16, tag=f"ghT{ht}", name=f"ghT{ht}") for ht in range(HT)]

            # collective DRAM buffers
            ib = [dram.tile([8, 128, 128], BF16, tag=f"ib{j}", name=f"ib{j}") for j in range(4)]
            ob = [dram.tile([8, 128, 128], BF16, tag=f"ob{j}", name=f"ob{j}") for j in range(4)]
            ib3w = [dram.tile([8, 64, 128], BF16, tag=f"ib3w{xx}", name=f"ib3w{xx}")
                    for xx in range(2)]
            ob3w = [dram.tile([8, 64, 128], BF16, tag=f"ob3w{xx}", name=f"ob3w{xx}")
                    for xx in range(2)]

            # ---------- helpers ----------
            def load_xta(jj, b):
                ch = 4 * b + jj
                xh = []
                for hh in range(2):
                    xth = wk.tile([128, 4 * 512], BF16, tag="xt", bufs=5, name="xta")
                    nc.sync.dma_start(xth[:], xT8[ch, :, 2048 * hh:2048 * (hh + 1)])
                    xh.append(xth)
                return xh

            def emit_chunk(jj, b, xta=None):
                """LN1 (folded post-matmul) + qkv (transposed) for chunk (b, jj)."""
                if xta is None:
                    xta = load_xta(jj, b)
                xt = [xta[pt // 4][:, 512 * (pt % 4):512 * (pt % 4 + 1)]
                      for pt in range(CB)]
                st1 = ps.tile([1, 512], F32, tag="po", bufs=2)
                st2 = ps.tile([1, 512], F32, tag="po", bufs=2)
                for pt in range(CB):
                    nc.tensor.matmul(st1[:], ones_bf[:], xt[pt],
                                     start=(pt == 0), stop=(pt == CB - 1))
                for h in range(2):
                    sqa = wk.tile([128, 4 * 512], BF16, tag="sq", bufs=1, name="sqa")
                    nc.scalar.activation(sqa[:], xta[h][:], AF.Square)
                    for pp_i in range(4):
                        pt = 4 * h + pp_i
                        nc.tensor.matmul(st2[:], ones_bf[:],
                                         sqa[:, 512 * pp_i:512 * (pp_i + 1)],
                                         start=(pt == 0), stop=(pt == CB - 1))
                mu2 = wk.tile([1, 512], F32, tag="arow", bufs=2, name="mu2")
                nc.scalar.activation(mu2[:], st1[:], AF.Square)
                var = wk.tile([1, 512], F32, tag="arow", bufs=2, name="var")
                nc.vector.tensor_tensor(var[:], st2[:], mu2[:], op=OP.subtract)
                rs_r = wk.tile([1, 512], BF16, tag="rsam", bufs=2, name="rs_r")
                nc.scalar.activation(rs_r[:], var[:], AF.Abs_reciprocal_sqrt,
                                     bias=eps1[:])
                am_r = wk.tile([1, 512], BF16, tag="rsam", bufs=2, name="am_r")
                nc.vector.tensor_tensor(am_r[:], st1[:], rs_r[:], op=OP.mult)
                bc_rs = wk.tile([128, 512], BF16, tag="bc", bufs=2, name="bc_rs")
                bc_a = wk.tile([128, 512], BF16, tag="bc", bufs=2, name="bc_a")
                nc.gpsimd.partition_broadcast(bc_rs[:], rs_r[:])
                nc.gpsimd.partition_broadcast(bc_a[:], am_r[:])

                wsb = wqk_sb if b == 0 else wqk2_sb

                def qk_mms(psum, base):
                    for pt in range(CB):
                        nc.tensor.matmul(psum, wsb[pt][:, base:base + 128],
                                         xt[pt], start=(pt == 0),
                                         stop=(pt == CB - 1))

                def fold_qk(dst_pair, gp, cs_ap, b_ap):
                    # dst = rs*G - (bc_a*cs - b), written per partition half
                    m = wk.tile([128, 512], BF16, tag="fold", bufs=3, name="m")
                    nc.vector.tensor_scalar(m[:], bc_a[:], cs_ap, b_ap,
                                            op0=OP.mult, op1=OP.subtract)
                    p1 = wk.tile([128, 512], BF16, tag="fold", bufs=3, name="p1")
                    nc.vector.tensor_tensor(p1[:], gp, bc_rs[:], op=OP.mult)
                    js = slice(512 * jj, 512 * (jj + 1))
                    nc.vector.tensor_tensor(dst_pair[0][0:64, js], p1[0:64, :],
                                            m[0:64, :], op=OP.subtract)
                    nc.vector.tensor_tensor(dst_pair[1][64:128, js], p1[64:128, :],
                                            m[64:128, :], op=OP.subtract)

                dq = (qt[0], qt[1]) if b == 0 else (qt[1], qt[0])
                dk = (kt_[0], kt_[1]) if b == 0 else (kt_[1], kt_[0])
                qk2 = ps.tile([128, 1024], F32, tag="w2", bufs=3)
                qk_mms(qk2[:, 0:512], 0)
                qk_mms(qk2[:, 512:1024], 128)
                fold_qk(dq, qk2[:, 0:512], cq_sb[b], bq_sb[b])
                fold_qk(dk, qk2[:, 512:1024], ck_sb[b], bk_sb[b])
                # v (transposed) then per-128 transpose into row-layout v1big
                pv2 = ps.tile([128, 1024], F32, tag="w2", bufs=3)
                pv = pv2[:, 0:512]
                for pt in range(CB):
                    nc.tensor.matmul(pv, wv_sb[pt], xt[pt],
                                     start=(pt == 0), stop=(pt == CB - 1))
                vts = wk.tile([128, 512], BF16, tag="vts", bufs=2)
                m = wk.tile([128, 512], BF16, tag="fold", bufs=3, name="m")
                nc.vector.tensor_scalar(m[:], bc_a[:], cv_sb, bv_sb,
                                        op0=OP.mult, op1=OP.subtract)
                p1 = wk.tile([128, 512], BF16, tag="fold", bufs=3, name="p1")
                nc.vector.tensor_tensor(p1[:], pv, bc_rs[:], op=OP.mult)
                nc.vector.tensor_tensor(vts[:], p1[:], m[:], op=OP.subtract)
                def fin(jj=jj, b=b, vts=vts):
                    ptr4 = ps.tile([128, 512], BF16, tag="po", bufs=2)
                    for t in range(4):
                        nc.tensor.transpose(ptr4[:, 128 * t:128 * (t + 1)],
                                            vts[:, 128 * t:128 * (t + 1)], idn[:])
                    dst = v1big[b].rearrange("p (k h d) -> p k h d", k=16, h=2)
                    nc.vector.tensor_copy(
                        dst[:, 4 * jj:4 * jj + 4, :, 0:64],
                        ptr4[:].rearrange("p (k h d) -> p k h d", k=4, h=2))
                flush_tr()
                pend[0] = fin

            pend = [None]

            def flush_tr():
                if pend[0] is not None:
                    pend[0]()
                    pend[0] = None

            def emit_attn(jj):
                """Causal attention for q-chunk jj (both heads X, both batches).

                Off-diagonal key tiles run full-width in exp-batched pairs;
                diagonal tiles use causal free-range slicing (queries >= 128*mrel)
                with only the boundary 128-col block masked."""
                nkt = 4 * jj + 4
                for X in range(2):
                    for u in range(2):
                        b = u if X == 0 else 1 - u
                        po_t = ps.tile([65, 512], F32, tag="po", bufs=2)
                        for mb in range(2 * jj):
                            s2 = ps.tile([128, 1024], F32, tag="w2", bufs=3)
                            for i in range(2):
                                kti = 2 * mb + i
                                nc.tensor.matmul(
                                    s2[:, 512 * i:512 * (i + 1)],
                                    kt_[X][64 * u:64 * (u + 1), 128 * kti:128 * (kti + 1)],
                                    qt[X][64 * u:64 * (u + 1), 512 * jj:512 * (jj + 1)],
                                    start=True, stop=True)
                            pt_sb = wk.tile([128, 1024], BF16, tag="ptb", bufs=2, name="pt_sb")
                            nc.scalar.activation(pt_sb[:], s2[:], AF.Exp)
                            for i in range(2):
                                kti = 2 * mb + i
                                sl = pt_sb[:, 512 * i:512 * (i + 1)]
                                nc.tensor.matmul(
                                    po_t[:],
                                    v1big[b][:, 130 * kti + 65 * X:130 * kti + 65 * X + 65],
                                    sl, start=(kti == 0), stop=False)
                        for mrel in range(4):
                            kti = 4 * jj + mrel
                            lo = 128 * mrel
                            w = 512 - lo
                            sd = ps.tile([128, 1024], F32, tag="w2", bufs=3)
                            nc.tensor.matmul(
                                sd[:, 0:w],
                                kt_[X][64 * u:64 * (u + 1), 128 * kti:128 * (kti + 1)],
                                qt[X][64 * u:64 * (u + 1),
                                      512 * jj + lo:512 * (jj + 1)],
                                start=True, stop=True)
                            ptd = wk.tile([128, 512], BF16, tag="ptb", bufs=2, name="ptd")
                            nc.scalar.activation(ptd[:, 0:w], sd[:, 0:w], AF.Exp)
                            nc.vector.tensor_tensor(ptd[:, 0:128], ptd[:, 0:128],
                                                    msk[mrel][:, lo:lo + 128],
                                                    op=OP.mult)
                            nc.tensor.matmul(
                                po_t[:, lo:512],
                                v1big[b][:, 130 * kti + 65 * X:130 * kti + 65 * X + 65],
                                ptd[:, 0:w],
                                start=(kti == 0), stop=(kti == nkt - 1))
                        dcp = wk.tile([1, 512], F32, tag="dcp", bufs=2, name="dcp")
                        nc.vector.tensor_copy(dcp[:], po_t[64:65, :])
                        recip = wk.tile([1, 512], F32, tag="dcp", bufs=2)
                        nc.vector.reciprocal_approx_fast(recip[:], dcp[:])
                        bcp = wk.tile([64, 512], F32, tag="bcb", bufs=1, name="bcp")
                        nc.gpsimd.partition_broadcast(bcp[:], recip[:])
                        yt = wk.tile([64, 512], BF16, tag="yt", bufs=1)
                        nc.vector.tensor_tensor(yt[:], po_t[0:64, :], bcp[:], op=OP.mult)
                        if jj == 3:
                            nc.gpsimd.dma_start(
                                ib3w[X][:, :, 64 * b:64 * b + 64]
                                .rearrange("c p t -> p c t"),
                                yt[:].rearrange("p (c t) -> p c t", c=8))
                            if u == 1:
                                nc.gpsimd.collective_compute(
                                    "AllToAll", OP.bypass,
                                    ins=[ib3w[X].opt()], outs=[ob3w[X].opt()],
                                    replica_groups=[list(range(NCORES))],
                                )
                        else:
                            nc.gpsimd.dma_start(
                                ib[jj][:, 64 * X:64 * X + 64, 64 * b:64 * b + 64]
                                .rearrange("c p t -> p c t"),
                                yt[:].rearrange("p (c t) -> p c t", c=8))
                if jj != 3:
                    nc.gpsimd.collective_compute(
                        "AllToAll", OP.bypass,
                        ins=[ib[jj].opt()], outs=[ob[jj].opt()],
                        replica_groups=[list(range(NCORES))],
                    )

            def emit_assembly(jj):
                """Scatter A2A output ob[jj] into yTbig columns."""
                dst = yTbig.rearrange("p (s g b t) -> p s g b t", s=8, g=4, b=2)
                for b in range(2):
                    if jj == 3:
                        for X in range(2):
                            nc.gpsimd.dma_start(
                                dst[64 * X:64 * X + 64, :, jj, b, :],
                                ob3w[X][:, :, 64 * b:64 * b + 64]
                                .rearrange("s p t -> p s t"))
                    else:
                        nc.gpsimd.dma_start(
                            dst[:, :, jj, b, :],
                            ob[jj][:, :, 64 * b:64 * b + 64].rearrange("s p t -> p s t"))

            def emit_proj(tt):
                """proj + residual for token tile tt (128 own rows)."""
                pps = ps.tile([128, 1024], F32, tag="w2", bufs=3)
                for nh in range(2):
                    dst = pps[:, 512 * nh:512 * (nh + 1)]
                    nc.tensor.matmul(dst, ones_row[0:1, :],
                                     bproj_sb[0:1, 512 * nh:512 * (nh + 1)],
                                     start=True, stop=False)
                    for cbh in range(2):
                        q = 2 * nh + cbh
                        for cbl in range(4):
                            cb = 4 * cbh + cbl
                            nc.tensor.matmul(
                                dst,
                                yTbig[:, 512 * cb + 128 * tt:512 * cb + 128 * (tt + 1)],
                                wpj[q][:, 512 * cbl:512 * (cbl + 1)],
                                start=False, stop=(cb == CB - 1))
                nc.vector.tensor_tensor(x2[tt], pps[:], x2[tt], op=OP.add)

            def emit_ln2(tt):
                """LN2 + transpose into ln2Tbig for token tile tt."""
                s1 = wk.tile([128, 1], F32, tag="e_s1")
                nc.vector.reduce_sum(s1[:], x2[tt], axis=AX.X)
                nmu = wk.tile([128, 1], F32, tag="e_nmu")
                nc.vector.tensor_scalar(nmu[:], s1[:], -1.0 / C, None, op0=OP.mult)
                sqs = wk.tile([128, C], F32, tag="sq", bufs=1, name="sqs")
                s2_ = wk.tile([128, 1], F32, tag="e_s2")
                nc.scalar.activation(sqs[:], x2[tt], AF.Square, accum_out=s2_[:])
                m2 = wk.tile([128, 1], F32, tag="e_m2")
                nc.vector.tensor_tensor(m2[:], nmu[:], nmu[:], op=OP.mult)
                var_ = wk.tile([128, 1], F32, tag="e_var")
                nc.vector.tensor_scalar(var_[:], s2_[:], 1.0 / C, EPS,
                                        op0=OP.mult, op1=OP.add)
                nc.vector.tensor_tensor(var_[:], var_[:], m2[:], op=OP.subtract)
                ti2 = wk.tile([128, 1], F32, tag="e_sd")
                nc.vector.tensor_scalar(ti2[:].bitcast(I32), var_[:].bitcast(I32),
                                        1, None, op0=OP.logical_shift_right)
                y02 = wk.tile([128, 1], F32, tag="e_y0")
                nc.vector.tensor_scalar(y02[:].bitcast(I32), ti2[:].bitcast(I32),
                                        -1, QK3, op0=OP.mult, op1=OP.add)
                aq2 = wk.tile([128, 1], F32, tag="e_aq")
                nc.vector.tensor_tensor(aq2[:], y02[:], y02[:], op=OP.mult)
                bq3 = wk.tile([128, 1], F32, tag="e_bq")
                nc.vector.tensor_tensor(bq3[:], aq2[:], var_[:], op=OP.mult)
                uq2 = wk.tile([128, 1], F32, tag="e_uq")
                nc.vector.tensor_scalar(uq2[:], bq3[:], -0.5, 1.5,
                                        op0=OP.mult, op1=OP.add)
                rs2 = wk.tile([128, 1], F32, tag="e_rs2")
                nc.vector.tensor_tensor(rs2[:], y02[:], uq2[:], op=OP.mult)
                na = wk.tile([128, 1], F32, tag="e_na")
                nc.vector.tensor_tensor(na[:], nmu[:], rs2[:], op=OP.mult)
                lr = wk.tile([128, C], BF16, tag="e_lr", bufs=1)
                nc.scalar.activation(lr[:], x2[tt], AF.Identity,
                                     bias=na[:], scale=rs2[:])
                ldst = ln2Tbig.rearrange("p (cb t) -> p cb t", cb=8)
                for hh in range(2):
                    ptr4 = ps.tile([128, 512], BF16, tag="po", bufs=2)
                    for t in range(4):
                        cb = 4 * hh + t
                        nc.tensor.transpose(ptr4[:, 128 * t:128 * (t + 1)],
                                            lr[:, 128 * cb:128 * (cb + 1)], idn[:])
                    nc.vector.tensor_copy(
                        ldst[:, 4 * hh:4 * hh + 4, 128 * tt:128 * (tt + 1)],
                        ptr4[:].rearrange("p (k t) -> p k t", k=4))

            def load_wfc(htg, eng):
                wq = []
                for qq_ in range(4):
                    wqt = wk.tile([128, 2 * 512], BF16, tag="wfc", bufs=8, name="wqt")
                    eng.dma_start(wqt[:], wfc[htg, :, 1024 * qq_:1024 * (qq_ + 1)])
                    wq.append(wqt)
                return wq

            def emit_fc(half, pre0=None):
                """fc + gelu for token half (0: tt0/tt1, 1: tt2/tt3)."""
                cs = slice(256 * half, 256 * (half + 1))
                for htg in range(8):
                    wq = pre0 if (htg == 0 and pre0 is not None) \
                        else load_wfc(htg, nc.sync)
                    for sub in range(4):
                        ht = 4 * htg + sub
                        ph = ps.tile([128, 256], F32, tag="w2", bufs=3)
                        for cb in range(CB):
                            wt = wq[cb // 2]
                            cbl = cb % 2
                            nc.tensor.matmul(
                                ph[:], wt[:, 512 * cbl + 128 * sub:512 * cbl + 128 * (sub + 1)],
                                ln2Tbig[:, 512 * cb + 256 * half:512 * cb + 256 * (half + 1)],
                                start=(cb == 0), stop=(cb == CB - 1))
                        nc.scalar.activation(ghT[ht][:, cs], ph[:], AF.Gelu,
                                             bias=bfc_sb[:, ht:ht + 1])

            # =================== main pipeline ===================
            # phase 1: all qkv chunks (Scalar holds the rsqrt table set)
            emit_chunk(0, 0)
            emit_chunk(0, 1)
            emit_chunk(1, 0)
            emit_chunk(1, 1)
            emit_chunk(2, 0)
            emit_chunk(2, 1)
            emit_chunk(3, 0)
            emit_chunk(3, 1)
            # big row-parallel loads: during attention (HBM is free then)
            nc.gpsimd.dma_start(x2big[:], x2r[:])
            nc.gpsimd.dma_start(wpjbig[:], wproj[:])
            flush_tr()
            # phase 2: attention (Scalar holds the exp table set)
            emit_attn(0)
            emit_attn(1)
            emit_attn(2)
            emit_assembly(0)
            emit_proj(0)
            emit_ln2(0)
            emit_assembly(1)
            emit_proj(1)
            emit_ln2(1)
            wq0 = load_wfc(0, nc.scalar)   # lands during attn(3)
            emit_attn(3)
            # phase 3: MLP (gelu set); fc pass 1 fills while the A2As fly
            emit_fc(0, pre0=wq0)
            emit_assembly(2)
            emit_proj(2)
            emit_ln2(2)
            emit_assembly(3)
            emit_proj(3)
            emit_ln2(3)
            emit_fc(1)

            # =========== MLP: fc_proj + residual -> output rows ===========
            pg = {}
            for tt in range(3):
                pg[tt] = ps.tile([128, 1024], F32, tag="w2", bufs=3, name=f"pg{tt}")
            pg3a = ps.tile([128, 512], F32, tag="po", bufs=2, name="pg3a")
            pg3b = ps.tile([128, 512], F32, tag="po", bufs=2, name="pg3b")

            def pg_dst(tt, nh):
                if tt < 3:
                    return pg[tt][:, 512 * nh:512 * (nh + 1)]
                return pg3a[:] if nh == 0 else pg3b[:]

            # seed psum chains with the bias so the epilogue is Vector-only
            for tt in range(4):
                for nh in range(2):
                    nc.tensor.matmul(pg_dst(tt, nh), ones_row[0:1, :],
                                     bfcp_sb[0:1, 512 * nh:512 * (nh + 1)],
                                     start=True, stop=False)
            for ht in range(HT):
                w = wk.tile([128, 1024], BF16, tag="wfcp", bufs=5, name="wfcp")
                nc.sync.dma_start(w[:], wfcp[ht])
                for tt in range(4):
                    for nh in range(2):
                        nc.tensor.matmul(pg_dst(tt, nh),
                                         ghT[ht][:, 128 * tt:128 * (tt + 1)],
                                         w[:, 512 * nh:512 * (nh + 1)],
                                         start=False, stop=(ht == HT - 1))
            for tt in range(4):
                for nh in range(2):
                    orow = wk.tile([128, 512], F32, tag="orow", bufs=1, name="orow")
                    nc.vector.tensor_tensor(
                        orow[:], pg_dst(tt, nh),
                        x2[tt][:, 512 * nh:512 * (nh + 1)], op=OP.add)
                    nc.sync.dma_start(out_rows[tt, :, 512 * nh:512 * (nh + 1)], orow[:])

    nc.compile()
    return nc


def _own_rows(c):
    rows = []
    for g in range(8):
        jjg, b = divmod(g, 2)
        base = 2048 * b + 512 * jjg + 64 * c
        rows.append(np.arange(base, base + 64))
    return np.concatenate(rows)


def _prep(inputs):
    """Host-side sharding/layout prep. Returns in_maps for the 8 cores."""
    f32 = np.float32
    bf = ml_dtypes.bfloat16
    x = np.asarray(inputs["x"], f32)
    ln1_w = np.asarray(inputs["ln1_w"], f32)
    ln1_b = np.asarray(inputs["ln1_b"], f32)
    attn_w = np.asarray(inputs["attn_w"], f32)
    attn_b = np.asarray(inputs["attn_b"], f32)
    proj_w = np.asarray(inputs["proj_w"], f32)
    proj_b = np.asarray(inputs["proj_b"], f32)
    ln2_w = np.asarray(inputs["ln2_w"], f32)
    ln2_b = np.asarray(inputs["ln2_b"], f32)
    fc_w = np.asarray(inputs["fc_w"], f32)
    fc_b = np.asarray(inputs["fc_b"], f32)
    fc_proj_w = np.asarray(inputs["fc_proj_w"], f32)
    fc_proj_b = np.asarray(inputs["fc_proj_b"], f32)

    # fold LN affine params into the following matmuls (exact linear identities)
    aw = ln1_w[:, None] * attn_w
    ab = ln1_b @ attn_w + attn_b
    fw = ln2_w[:, None] * fc_w
    fb = ln2_b @ fc_w + fc_b

    sc = 1.0 / np.sqrt(HD)
    xg = x.reshape(BT, C)                                  # global token rows
    # xT8[ch, p, 512*pt + q] = x_g[512*ch + q, 128*pt + p]
    xT8 = np.ascontiguousarray(
        xg.reshape(NCH, 512, CB, 128).transpose(0, 3, 2, 1).reshape(NCH, 128, 8 * 512)
    ).astype(bf)
    # wproj[p, 2048*(2nh+cbh) + 512*cbl + j] = proj_w[128*(4*cbh+cbl) + p, 512*nh + j]
    wproj_h = np.ascontiguousarray(
        proj_w.reshape(2, 4, 128, 2, 512).transpose(2, 3, 0, 1, 4)
        .reshape(128, 4 * 2048)).astype(bf)
    # wfc[htg][p, 512*cb + 128*sub + j] = fw[128*cb + p, 512*htg + 128*sub + j]
    wfc_h = np.ascontiguousarray(
        fw.reshape(CB, 128, 8, 512).transpose(2, 1, 0, 3).reshape(8, 128, 8 * 512)
    ).astype(bf)
    bfc_h = np.ascontiguousarray(fb.reshape(HT, 128).T).astype(f32)
    wfcp_h = np.ascontiguousarray(fc_proj_w.reshape(HT, 128, 1024)).astype(bf)

    maskd = np.zeros((4, 128, 512), np.float32)
    for m in range(4):
        maskd[m] = (128 * m + np.arange(128)[:, None]) <= np.arange(512)[None, :]
    maskd = np.ascontiguousarray(maskd.transpose(1, 0, 2).reshape(128, 4 * 512)).astype(bf)
    ident = np.eye(128, dtype=np.float32).astype(bf)

    shared = dict(
        xT8=xT8,
        wproj=wproj_h,
        wfc=wfc_h,
        wfcp=wfcp_h,
        maskd=maskd, ident=ident,
    )
    rpack = np.concatenate(
        [np.ones(128, f32), proj_b.astype(f32), fc_proj_b.astype(f32)]
    ).reshape(1, 128 + 2 * C)

    in_maps = []
    for c in range(NCORES):
        h0 = 2 * c
        qcols = aw[:, 64 * h0:64 * h0 + 128] * sc          # [1024, 128] both heads' q
        kcols = aw[:, C + 64 * h0:C + 64 * h0 + 128]
        vcols = aw[:, 2 * C + 64 * h0:2 * C + 64 * h0 + 128]
        wqk_full = np.concatenate([qcols, kcols], axis=1)  # [1024, 256]

        def sw(m):  # swap 64-col head halves within each 128 block
            return np.concatenate([m[:, 64:128], m[:, 0:64]], axis=1)

        wqk2_full = np.concatenate([sw(qcols), sw(kcols)], axis=1)
        wqka_c = np.ascontiguousarray(
            wqk_full.reshape(CB, 128, 256).transpose(1, 0, 2).reshape(128, CB * 256)
        ).astype(bf)
        wqkb_c = np.ascontiguousarray(
            wqk2_full.reshape(CB, 128, 256).transpose(1, 0, 2).reshape(128, CB * 256)
        ).astype(bf)
        wva_c = np.ascontiguousarray(
            vcols.reshape(CB, 128, 128).transpose(1, 0, 2).reshape(128, CB * 128)
        ).astype(bf)

        bq = ab[64 * h0:64 * h0 + 128] * sc
        bk = ab[C + 64 * h0:C + 64 * h0 + 128]
        wqk_q = wqk_full.astype(bf).astype(f32)            # bf16-quantized col sums
        cq = wqk_q[:, 0:128].sum(axis=0)
        ck = wqk_q[:, 128:256].sum(axis=0)

        def swv(v):  # [h0 h1](64 each) -> [h1 h0]
            return np.concatenate([v[64:], v[:64]])

        bv_c = ab[2 * C + 64 * h0:2 * C + 64 * h0 + 128]
        cv_c = vcols.astype(bf).astype(f32).sum(axis=0)
        cpack_c = np.stack(
            [bq, swv(bq), bk, swv(bk), cq, swv(cq), ck, swv(ck), bv_c, cv_c],
            axis=1).astype(f32)                            # [128, 10]
        cpack_c = np.concatenate([cpack_c, bfc_h], axis=1)  # [128, 42]

        x2r_c = np.ascontiguousarray(
            xg[_own_rows(c)].reshape(4, 128, C).transpose(1, 0, 2).reshape(128, 4 * C)
        ).astype(f32)
        m = dict(shared)
        m.update(wqka=wqka_c, wqkb=wqkb_c, wva=wva_c, cpack=cpack_c,
                 rpack=rpack, x2r=x2r_c)
        in_maps.append(m)
    return in_maps


def kernel(**inputs) -> np.ndarray:
    if "nc" not in _cache:
        _cache["nc"] = build()
    nc = _cache["nc"]
    in_maps = _prep(inputs)
    res = bass_utils.run_bass_kernel_spmd(nc, in_maps, core_ids=list(range(NCORES)))
    out = np.empty((BT, C), np.float32)
    for c in range(NCORES):
        out[_own_rows(c)] = res.results[c]["out_rows"].reshape(RPC, C)
    return out.reshape(B, T, C)
